# revision 15
# baseline (speedup 1.0000x reference)
"""Trainium2 Bass kernel for DeepICD candidate attention.

Reference computation (per batch b):
    S     = X[b] @ a_w                      [L, OS]     (a_b drops out of softmax)
    alpha = softmax(S, axis=L)
    Xp    = alpha^T @ X[b]                  [OS, D]
    Xph   = Xp @ hw_eff + hb_eff            [OS, LAB]   (BN folded into hw/hb on host)
    Xpf   = relu(Xph)
    bLV   = labDescVec[candidate[b]]        [NC, LAB]
    sc    = Xpf @ bLV^T                     [OS, NC]
    a2    = softmax(sc, axis=OS)
    out   = a2^T @ Xpf                      [NC, LAB]

Sharding: data-parallel over batch B=16 across 8 NeuronCores (2 batches/core);
weights and labDescVec replicated.  X is cast to bf16 on the host so the
device loads ride plain HWDGE DMAs at half the bytes (no SWDGE cast stream).

Per-core schedule:
  phase A processes the two batches interleaved per l-tile; the two Xp
  accumulations are column-tiled (batch0 -> PSUM partitions 0-63, batch1 ->
  64-127) so the two M=64 matmuls share the PE array.  The stacked [128, D]
  Xp then runs phase B (h-projection) at full M=128 once for both batches.
  Phase D's two K=64 output matmuls are row-tiled (rows 0-63 / 64-127).

softmax over L is computed without max subtraction (S ~ N(0,1), |S| < ~6, exp
is safe in fp32) so the L-dim reduction becomes a matmul with a ones vector.
"""

import numpy as np

P = 128
NB = 2          # batches per core
L = 2048
D = 1024
OS = 64
NCC = 256       # candidates per sample
LAB = 1024
CLS = 8921
NT = L // P     # 16 l-tiles
DC = D // P     # 8 d-chunks
HC = LAB // P   # 8 h-chunks
CC = NCC // P   # 2 candidate chunks
N_CORES = 8
BN_EPS = 1e-5

_PROG = None


def _build_program():
    import concourse.bass as bass
    import concourse.bacc as bacc
    import concourse.tile as tile
    from concourse import mybir
    from concourse.masks import make_identity

    f32 = mybir.dt.float32
    bf16 = mybir.dt.bfloat16
    i32 = mybir.dt.int32
    AF = mybir.ActivationFunctionType

    nc = bacc.Bacc("TRN2", target_bir_lowering=False, debug=False,
                   num_devices=N_CORES)
    X = nc.dram_tensor("X", [NB, L, D], bf16, kind="ExternalInput")
    cand = nc.dram_tensor("cand", [NB, NCC], i32, kind="ExternalInput")
    aw = nc.dram_tensor("aw", [D, OS], bf16, kind="ExternalInput")
    hw = nc.dram_tensor("hw", [D, LAB], bf16, kind="ExternalInput")
    hb = nc.dram_tensor("hb", [LAB], bf16, kind="ExternalInput")
    lab = nc.dram_tensor("lab", [CLS, LAB], bf16, kind="ExternalInput")
    out_d = nc.dram_tensor("out", [NB, NCC, LAB], bf16, kind="ExternalOutput")

    with tile.TileContext(nc) as tc:
        with (
            tc.tile_pool(name="singles", bufs=1) as singles,
            tc.tile_pool(name="gat", bufs=NB * CC) as gat,
            tc.tile_pool(name="xin", bufs=8) as xin,
            tc.tile_pool(name="work", bufs=2) as work,
            tc.tile_pool(name="outp", bufs=4) as outp,
            tc.tile_pool(name="pa", bufs=2, space="PSUM") as pa,
            tc.tile_pool(name="pb", bufs=3, space="PSUM") as pb,
            tc.tile_pool(name="pacc", bufs=1, space="PSUM") as pacc,
        ):
            # ---- PE warm-up burst: dummy matmuls while the first X tiles are
            # in flight, so HAM un-throttles (4/8 -> 8/8) before phase A.
            # Transpose-mode ops don't count as PE-busy for HAM; matmuls do.
            scratch = singles.tile([P, P], bf16)
            nc.vector.memset(scratch[:], 1.0)
            warm_ps = pa.tile([P, P], f32, tag="tp")
            for _ in range(24):
                nc.tensor.matmul(
                    out=warm_ps[:], lhsT=scratch[:], rhs=scratch[:],
                    start=True, stop=True,
                )

            xbf = {}

            def load_x(ii):
                for b in range(NB):
                    t_ = xin.tile([P, 2, D], bf16, tag="xbf")
                    nc.sync.dma_start(
                        out=t_[:],
                        in_=X[b, ii * 2 * P:(ii + 1) * 2 * P, :].rearrange(
                            "(t p) d -> p t d", p=P
                        ),
                    )
                    xbf[b, ii] = t_

            # X tiles lead the sync ring so phase A starts as early as possible
            ident = singles.tile([P, P], bf16)
            aw_bf = singles.tile([P, DC, OS], bf16)
            t0 = xin.tile([P, 2, D], bf16, tag="xbf")
            nc.sync.dma_start(
                out=t0[:],
                in_=X[0, 0:2 * P, :].rearrange("(t p) d -> p t d", p=P),
            )
            xbf[0, 0] = t0
            nc.sync.dma_start(
                out=aw_bf[:], in_=aw[:, :].rearrange("(c p) o -> p c o", p=P)
            )
            t1 = xin.tile([P, 2, D], bf16, tag="xbf")
            nc.sync.dma_start(
                out=t1[:],
                in_=X[1, 0:2 * P, :].rearrange("(t p) d -> p t d", p=P),
            )
            xbf[1, 0] = t1
            load_x(1)
            make_identity(nc, ident[:])
            ones_col = singles.tile([P, 1], bf16)
            nc.vector.memset(ones_col[:], 1.0)
            ones_one = singles.tile([1, P], bf16)
            nc.vector.memset(ones_one[:], 1.0)
            cand_sb = singles.tile([P, NB, CC], i32)
            nc.sync.dma_start(
                out=cand_sb[:], in_=cand[:, :].rearrange("b (c p) -> p b c", p=P)
            )
            hb_bf = singles.tile([1, LAB], bf16)
            nc.sync.dma_start(out=hb_bf[:], in_=hb[None, :])
            hw_bf = singles.tile([P, DC, LAB], bf16)

            # phase-A accumulators: batch0 on partitions 0-63, batch1 on 64-127.
            # The softmax partition function accumulates as a ROW [1, o2] so it
            # can later feed the rank-1 zz*hb bias matmul (normalize-late).
            xpu2 = pacc.tile([P, D], f32, tag="xpu2")
            zzT = pacc.tile([1, P], f32, tag="zzT")

            blv_f = {}

            def emit_xp(info):
                # column-tiled pair: b0 -> out partitions 0-63 (tile (0,0)),
                # b1 -> 64-127 (tile (0,64)); adjacent queue slots run
                # concurrently on the two PE column groups
                i_, es = info
                ii_, t_ = i_ // 2, i_ % 2
                for nh in range(2):
                    for b in range(NB):
                        nc.tensor.matmul(
                            out=xpu2[b * OS:(b + 1) * OS,
                                     nh * 512:(nh + 1) * 512],
                            lhsT=es[b][:],
                            rhs=xbf[b, ii_][:, t_, nh * 512:(nh + 1) * 512],
                            start=(i_ == 0), stop=(i_ == NT - 1),
                            skip_group_check=True,
                        )
                for b in range(NB):
                    nc.tensor.matmul(
                        out=zzT[0:1, b * OS:(b + 1) * OS],
                        lhsT=ones_col[:], rhs=es[b][:],
                        start=(i_ == 0), stop=(i_ == NT - 1),
                        skip_group_check=True,
                    )

            # ======== phase A: attention pooling over L, batches interleaved ==
            prev = None
            for i in range(NT):
                ii, t = i // 2, i % 2
                if i % 2 == 0 and i // 2 + 2 < NT // 2:
                    load_x(i // 2 + 2)
                xts = {}
                for b in range(NB):
                    # X tile transposed (d on partitions) for the S matmul
                    tp = pa.tile([P, DC, P], bf16, tag="tp")
                    for c in range(DC):
                        nc.tensor.transpose(
                            out=tp[:, c, :],
                            in_=xbf[b, ii][:, t, c * P:(c + 1) * P],
                            identity=ident[:],
                        )
                    xt_sb = work.tile([P, DC, P], bf16, tag="xt", bufs=3)
                    nc.vector.tensor_copy(out=xt_sb[:], in_=tp[:])
                    xts[b] = xt_sb
                es = {}
                for b in range(NB):
                    s_ps = pb.tile([P, OS], f32, tag="sps")
                    for c in range(DC):
                        nc.tensor.matmul(
                            out=s_ps[:], lhsT=xts[b][:, c, :],
                            rhs=aw_bf[:, c, :],
                            start=(c == 0), stop=(c == DC - 1),
                        )
                    e_sb = work.tile([P, OS], bf16, tag="e", bufs=4)
                    nc.scalar.activation(out=e_sb[:], in_=s_ps[:], func=AF.Exp)
                    es[b] = e_sb
                if prev is not None:
                    emit_xp(prev)
                prev = (i, es)
                if i == 8:
                    # candidate-row gathers on the otherwise-idle SWDGE path
                    for gb in range(NB):
                        for cc in range(CC):
                            bf_t = gat.tile([P, LAB], bf16, tag="blvf",
                                            name=f"blvf_{gb}_{cc}")
                            nc.gpsimd.indirect_dma_start(
                                out=bf_t[:], out_offset=None, in_=lab[:, :],
                                in_offset=bass.IndirectOffsetOnAxis(
                                    ap=cand_sb[:, gb, cc:cc + 1], axis=0,
                                ),
                            )
                            blv_f[gb, cc] = bf_t
                if i == 13:
                    # bulk h-weights land right before phase B needs them,
                    # queued on the sync ring behind the X stream
                    for j in range(4):
                        nc.sync.dma_start(
                            out=hw_bf[:, j * 2:(j + 1) * 2, :],
                            in_=hw[j * 2 * P:(j + 1) * 2 * P, :].rearrange(
                                "(c p) h -> p c h", p=P
                            ),
                        )
            emit_xp(prev)

            # candidate-vector transposes go first on the PE queue: they only
            # need the gathers, so the PE stays busy while the phase-B
            # normalize chain (reciprocal -> scale -> evac) runs on DVE/ACT
            blvT = {}
            for b in range(NB):
                bt = work.tile([P, HC, NCC], bf16, tag=f"blvT{b}")
                for cc in range(CC):
                    tp4 = pa.tile([P, HC, P], bf16, tag="tp")
                    for hc in range(HC):
                        nc.tensor.transpose(
                            out=tp4[:, hc, :],
                            in_=blv_f[b, cc][:, hc * P:(hc + 1) * P],
                            identity=ident[:],
                        )
                    nc.vector.tensor_copy(
                        out=bt[:, :, cc * P:(cc + 1) * P], in_=tp4[:]
                    )
                blvT[b] = bt

            # ======== phase B: project, both batches stacked ========
            # normalize-late: Xp stays unnormalized (values ~zz*Xp, fine in
            # bf16); the h-projection gets a rank-1 zz[o]*hb[h] bias and the
            # relu evacuation applies the per-partition 1/zz[o] scale.
            zz_row = work.tile([1, P], bf16, tag="zzrow")
            nc.vector.tensor_copy(out=zz_row[:], in_=zzT[:])
            zc_ps = pb.tile([P, 1], f32, tag="sps")
            nc.tensor.matmul(
                out=zc_ps[:], lhsT=zz_row[:], rhs=ones_one[0:1, 0:1],
                start=True, stop=True,
            )
            rzA = work.tile([P, 1], f32, tag="rzA")
            nc.vector.reciprocal(out=rzA[:], in_=zc_ps[:])
            xp2 = work.tile([P, D], bf16, tag="xp2")
            for nh in range(2):
                nc.vector.tensor_copy(
                    out=xp2[:, nh * 512:(nh + 1) * 512],
                    in_=xpu2[:, nh * 512:(nh + 1) * 512],
                )
            # Xp^T (d on partitions); evacuate in halves so the projection
            # matmuls start after the first four chunks
            xpt2 = work.tile([P, DC, P], bf16, tag="xpt2")
            tp2 = pa.tile([P, DC, P], bf16, tag="tp")
            for c in range(DC):
                nc.tensor.transpose(
                    out=tp2[:, c, :], in_=xp2[:, c * P:(c + 1) * P],
                    identity=ident[:],
                )
                if c == DC // 2 - 1:
                    nc.vector.tensor_copy(
                        out=xpt2[:, :DC // 2, :], in_=tp2[:, :DC // 2, :]
                    )
            nc.vector.tensor_copy(
                out=xpt2[:, DC // 2:, :], in_=tp2[:, DC // 2:, :]
            )
            xpf2 = work.tile([P, LAB], bf16, tag="xpf2")
            for nh in range(2):
                xph = pb.tile([P, 512], f32, tag="sps")
                for c in range(DC):
                    nc.tensor.matmul(
                        out=xph[:], lhsT=xpt2[:, c, :],
                        rhs=hw_bf[:, c, nh * 512:(nh + 1) * 512],
                        start=(c == 0), stop=False,
                    )
                nc.tensor.matmul(
                    out=xph[:], lhsT=zz_row[:],
                    rhs=hb_bf[:, nh * 512:(nh + 1) * 512],
                    start=False, stop=True,
                )
                nc.scalar.activation(
                    out=xpf2[:, nh * 512:(nh + 1) * 512], in_=xph[:],
                    func=AF.Relu, scale=rzA[:],
                )
            # Xpf^T (h on partitions) for the candidate scores
            xpft2 = work.tile([P, HC, P], bf16, tag="xpft2")
            tp3 = pa.tile([P, HC, P], bf16, tag="tp")
            for hc in range(HC):
                nc.tensor.transpose(
                    out=tp3[:, hc, :], in_=xpf2[:, hc * P:(hc + 1) * P],
                    identity=ident[:],
                )
                if hc == HC // 2 - 1:
                    nc.vector.tensor_copy(
                        out=xpft2[:, :HC // 2, :], in_=tp3[:, :HC // 2, :]
                    )
            nc.vector.tensor_copy(
                out=xpft2[:, HC // 2:, :], in_=tp3[:, HC // 2:, :]
            )

            # ======== phase C/D interleaved per candidate chunk ========
            # deferred softmax over OS: out_unnorm = E2^T Xpf, scaled by
            # 1/rowsum at PSUM evacuation
            e2t2 = work.tile([P, CC, P], bf16, tag="e2t2")
            tp5 = pa.tile([P, CC, P], bf16, tag="tp")
            rz2s = {}
            for cc in range(CC):
                for b in range(NB):
                    s2 = pb.tile([P, OS], f32, tag="sps")
                    for hc in range(HC):
                        nc.tensor.matmul(
                            out=s2[:],
                            lhsT=blvT[b][:, hc, cc * P:(cc + 1) * P],
                            rhs=xpft2[:, hc, b * OS:(b + 1) * OS],
                            start=(hc == 0), stop=(hc == HC - 1),
                        )
                    negm = work.tile([P, 1], f32, tag="negm")
                    nc.vector.tensor_reduce(
                        out=negm[:], in_=s2[:], axis=mybir.AxisListType.X,
                        op=mybir.AluOpType.max, negate=True,
                    )
                    e2 = work.tile([P, OS], bf16, tag="e2")
                    sume = work.tile([P, 1], f32, tag="sume")
                    nc.scalar.activation(
                        out=e2[:], in_=s2[:], func=AF.Exp, bias=negm[:],
                        accum_out=sume[:],
                    )
                    rz2 = work.tile([P, 1], f32, tag="rz2", bufs=4,
                                    name=f"rz2_{b}_{cc}")
                    nc.vector.reciprocal(out=rz2[:], in_=sume[:])
                    rz2s[b, cc] = rz2
                    # E2^T lands on partitions b*64..: batch1 goes to PE
                    # column group (0, 64); evacuate per quadrant so phase D
                    # for this cc can start immediately
                    nc.tensor.transpose(
                        out=tp5[b * OS:(b + 1) * OS, cc, :], in_=e2[:],
                        identity=ident[:],
                    )
                    nc.vector.tensor_copy(
                        out=e2t2[b * OS:(b + 1) * OS, cc, :],
                        in_=tp5[b * OS:(b + 1) * OS, cc, :],
                    )
            # out = softmax(s2)^T Xpf: K=64 row-tiled pairs, b0 on PE
            # rows 0-63, b1 on rows 64-127; the softmax chains for later
            # chunks drain on DVE/ACT underneath these matmuls
            for cc in range(CC):
                for nh in range(2):
                    ops = {}
                    for b in range(NB):
                        op = pb.tile([P, 512], f32, tag="sps")
                        nc.tensor.matmul(
                            out=op[:],
                            lhsT=e2t2[b * OS:(b + 1) * OS, cc, :],
                            rhs=xpf2[b * OS:(b + 1) * OS,
                                     nh * 512:(nh + 1) * 512],
                            start=True, stop=True,
                        )
                        ops[b] = op
                    for b in range(NB):
                        ob = outp.tile([P, 512], bf16, tag="ob")
                        if b == 0:
                            nc.scalar.activation(
                                out=ob[:], in_=ops[b][:], func=AF.Copy,
                                scale=rz2s[b, cc][:],
                            )
                        else:
                            nc.vector.tensor_scalar(
                                out=ob[:], in0=ops[b][:],
                                scalar1=rz2s[b, cc][:],
                                scalar2=None, op0=mybir.AluOpType.mult,
                            )
                        nc.sync.dma_start(
                            out=out_d[b, cc * P:(cc + 1) * P,
                                      nh * 512:(nh + 1) * 512],
                            in_=ob[:],
                        )
    nc.finalize()
    return nc


def _ensure_neuron_platform():
    # The kernel must execute on the axon-tunneled NeuronCores; a stray
    # JAX_PLATFORMS=cpu pin (common for running the jax reference) would
    # hide them from PJRT. Only act if jax hasn't initialized a backend yet.
    import os
    import sys

    if os.environ.get("JAX_PLATFORMS") == "cpu":
        jax = sys.modules.get("jax")
        initialized = False
        if jax is not None:
            try:
                from jax._src import xla_bridge

                initialized = xla_bridge.backends_are_initialized()
            except Exception:
                initialized = False
        if not initialized:
            del os.environ["JAX_PLATFORMS"]


def _get_program():
    global _PROG
    if _PROG is None:
        _ensure_neuron_platform()
        _PROG = _build_program()
    return _PROG


def _make_in_maps(inputs):
    import ml_dtypes

    bf16 = ml_dtypes.bfloat16
    X = np.ascontiguousarray(
        np.asarray(inputs["X"], dtype=np.float32).astype(bf16)
    )
    cand = np.ascontiguousarray(
        np.asarray(inputs["candidate"]).astype(np.int32)
    )
    a_w = np.asarray(inputs["a_w"], dtype=np.float32)
    h_w = np.asarray(inputs["h_w"], dtype=np.float32)
    h_b = np.asarray(inputs["h_b"], dtype=np.float32)
    g = np.asarray(inputs["bn_gamma"], dtype=np.float32)
    be = np.asarray(inputs["bn_beta"], dtype=np.float32)
    mu = np.asarray(inputs["bn_mean"], dtype=np.float32)
    var = np.asarray(inputs["bn_var"], dtype=np.float32)
    lab = np.ascontiguousarray(
        np.asarray(inputs["labDescVec"], dtype=np.float32).astype(bf16)
    )

    s = g / np.sqrt(var + BN_EPS)
    hw_eff = np.ascontiguousarray((h_w * s[None, :]).astype(bf16))
    hb_eff = ((h_b - mu) * s + be).astype(bf16)
    aw_bf = a_w.astype(bf16)

    in_maps = []
    for ci in range(N_CORES):
        in_maps.append({
            "X": X[ci * NB:(ci + 1) * NB],
            "cand": cand[ci * NB:(ci + 1) * NB],
            "aw": aw_bf,
            "hw": hw_eff,
            "hb": hb_eff,
            "lab": lab,
        })
    return in_maps


def run(inputs, trace=False, tmpdir=None):
    from concourse.bass_utils import run_bass_kernel_spmd

    nc = _get_program()
    in_maps = _make_in_maps(inputs)
    kwargs = {}
    if trace and tmpdir is None:
        tmpdir = "/root/problem/trace_out"
        import os
        import shutil

        shutil.rmtree(tmpdir, ignore_errors=True)
        os.makedirs(tmpdir, exist_ok=True)
    if tmpdir is not None:
        kwargs["tmpdir"] = tmpdir
    res = run_bass_kernel_spmd(
        nc, in_maps, list(range(N_CORES)), trace=trace, **kwargs,
    )
    out = np.concatenate(
        [np.asarray(r["out"]).astype(np.float32) for r in res.results], axis=0
    )
    return out, res


def kernel(**inputs):
    out, _ = run(inputs, trace=False)
    return out


# revision 19
# speedup vs baseline: 1.0773x; 1.0773x over previous
"""Trainium2 Bass kernel for DeepICD candidate attention.

Reference computation (per batch b):
    S     = X[b] @ a_w                      [L, OS]     (a_b drops out of softmax)
    alpha = softmax(S, axis=L)
    Xp    = alpha^T @ X[b]                  [OS, D]
    Xph   = Xp @ hw_eff + hb_eff            [OS, LAB]   (BN folded into hw/hb on host)
    Xpf   = relu(Xph)
    bLV   = labDescVec[candidate[b]]        [NC, LAB]
    sc    = Xpf @ bLV^T                     [OS, NC]
    a2    = softmax(sc, axis=OS)
    out   = a2^T @ Xpf                      [NC, LAB]

Sharding: data-parallel over batch B=16 across 8 NeuronCores (2 batches/core);
weights and labDescVec replicated.  X is cast to bf16 on the host so the
device loads ride plain HWDGE DMAs at half the bytes (no SWDGE cast stream).

Per-core schedule:
  phase A processes the two batches interleaved per l-tile; the two Xp
  accumulations are column-tiled (batch0 -> PSUM partitions 0-63, batch1 ->
  64-127) so the two M=64 matmuls share the PE array.  The stacked [128, D]
  Xp then runs phase B (h-projection) at full M=128 once for both batches.
  Phase D's two K=64 output matmuls are row-tiled (rows 0-63 / 64-127).

softmax over L is computed without max subtraction (S ~ N(0,1), |S| < ~6, exp
is safe in fp32) so the L-dim reduction becomes a matmul with a ones vector.
"""

import numpy as np

P = 128
NB = 2          # batches per core
L = 2048
D = 1024
OS = 64
NCC = 256       # candidates per sample
LAB = 1024
CLS = 8921
NT = L // P     # 16 l-tiles
DC = D // P     # 8 d-chunks
HC = LAB // P   # 8 h-chunks
CC = NCC // P   # 2 candidate chunks
N_CORES = 8
BN_EPS = 1e-5

_PROG = None


def _build_program():
    import concourse.bass as bass
    import concourse.bacc as bacc
    import concourse.tile as tile
    from concourse import mybir
    from concourse.masks import make_identity

    f32 = mybir.dt.float32
    bf16 = mybir.dt.bfloat16
    i32 = mybir.dt.int32
    AF = mybir.ActivationFunctionType

    nc = bacc.Bacc("TRN2", target_bir_lowering=False, debug=False,
                   num_devices=N_CORES)
    X = nc.dram_tensor("X", [NB, L, D], bf16, kind="ExternalInput")
    cand = nc.dram_tensor("cand", [NB, NCC], i32, kind="ExternalInput")
    aw = nc.dram_tensor("aw", [D, OS], bf16, kind="ExternalInput")
    hw = nc.dram_tensor("hw", [D, LAB], bf16, kind="ExternalInput")
    hb = nc.dram_tensor("hb", [LAB], bf16, kind="ExternalInput")
    lab = nc.dram_tensor("lab", [CLS, LAB], bf16, kind="ExternalInput")
    out_d = nc.dram_tensor("out", [NB, NCC, LAB], bf16, kind="ExternalOutput")

    with tile.TileContext(nc) as tc:
        with (
            tc.tile_pool(name="singles", bufs=1) as singles,
            tc.tile_pool(name="gat", bufs=NB * CC) as gat,
            tc.tile_pool(name="xin", bufs=8) as xin,
            tc.tile_pool(name="work", bufs=2) as work,
            tc.tile_pool(name="outp", bufs=4) as outp,
            tc.tile_pool(name="pa", bufs=2, space="PSUM") as pa,
            tc.tile_pool(name="pb", bufs=3, space="PSUM") as pb,
            tc.tile_pool(name="pacc", bufs=1, space="PSUM") as pacc,
        ):
            # ---- PE warm-up burst: dummy matmuls while the first X tiles are
            # in flight, so HAM un-throttles (4/8 -> 8/8) before phase A.
            # Transpose-mode ops don't count as PE-busy for HAM; matmuls do.
            scratch = singles.tile([P, P], bf16)
            nc.vector.memset(scratch[:], 1.0)
            warm_ps = pa.tile([P, P], f32, tag="tp")
            for _ in range(24):
                nc.tensor.matmul(
                    out=warm_ps[:], lhsT=scratch[:], rhs=scratch[:],
                    start=True, stop=True,
                )

            xbf = {}

            def load_x(ii):
                for b in range(NB):
                    t_ = xin.tile([P, 2, D], bf16, tag="xbf")
                    nc.sync.dma_start(
                        out=t_[:],
                        in_=X[b, ii * 2 * P:(ii + 1) * 2 * P, :].rearrange(
                            "(t p) d -> p t d", p=P
                        ),
                    )
                    xbf[b, ii] = t_

            # X tiles lead the sync ring so phase A starts as early as possible
            ident = singles.tile([P, P], bf16)
            aw_bf = singles.tile([P, DC, OS], bf16)
            t0 = xin.tile([P, 2, D], bf16, tag="xbf")
            nc.sync.dma_start(
                out=t0[:],
                in_=X[0, 0:2 * P, :].rearrange("(t p) d -> p t d", p=P),
            )
            xbf[0, 0] = t0
            nc.sync.dma_start(
                out=aw_bf[:], in_=aw[:, :].rearrange("(c p) o -> p c o", p=P)
            )
            t1 = xin.tile([P, 2, D], bf16, tag="xbf")
            nc.sync.dma_start(
                out=t1[:],
                in_=X[1, 0:2 * P, :].rearrange("(t p) d -> p t d", p=P),
            )
            xbf[1, 0] = t1
            load_x(1)
            make_identity(nc, ident[:])
            ones_col = singles.tile([P, 1], bf16)
            nc.vector.memset(ones_col[:], 1.0)
            ones_one = singles.tile([1, P], bf16)
            nc.vector.memset(ones_one[:], 1.0)
            cand_sb = singles.tile([P, NB, CC], i32)
            nc.sync.dma_start(
                out=cand_sb[:], in_=cand[:, :].rearrange("b (c p) -> p b c", p=P)
            )
            hb_bf = singles.tile([1, LAB], bf16)
            nc.sync.dma_start(out=hb_bf[:], in_=hb[None, :])
            hw_bf = singles.tile([P, DC, LAB], bf16)

            # phase-A accumulators: batch0 on partitions 0-63, batch1 on 64-127
            xpu2 = pacc.tile([P, D], f32, tag="xpu2")
            zz2 = pacc.tile([P, 1], f32, tag="zz2")

            blv_f = {}

            def emit_xp(info):
                # column-tiled pair: b0 -> out partitions 0-63 (tile (0,0)),
                # b1 -> 64-127 (tile (0,64)); adjacent queue slots run
                # concurrently on the two PE column groups
                i_, es = info
                ii_, t_ = i_ // 2, i_ % 2
                for nh in range(2):
                    for b in range(NB):
                        nc.tensor.matmul(
                            out=xpu2[b * OS:(b + 1) * OS,
                                     nh * 512:(nh + 1) * 512],
                            lhsT=es[b][:],
                            rhs=xbf[b, ii_][:, t_, nh * 512:(nh + 1) * 512],
                            start=(i_ == 0), stop=(i_ == NT - 1),
                            skip_group_check=True,
                        )
                for b in range(NB):
                    nc.tensor.matmul(
                        out=zz2[b * OS:(b + 1) * OS, :],
                        lhsT=es[b][:], rhs=ones_col[:],
                        start=(i_ == 0), stop=(i_ == NT - 1),
                        skip_group_check=True,
                    )

            # ======== phase A: attention pooling over L, batches interleaved ==
            prev = None
            for i in range(NT):
                ii, t = i // 2, i % 2
                if i % 2 == 0 and i // 2 + 2 < NT // 2:
                    load_x(i // 2 + 2)
                xts = {}
                for b in range(NB):
                    # X tile transposed (d on partitions) for the S matmul
                    tp = pa.tile([P, DC, P], bf16, tag="tp")
                    for c in range(DC):
                        nc.tensor.transpose(
                            out=tp[:, c, :],
                            in_=xbf[b, ii][:, t, c * P:(c + 1) * P],
                            identity=ident[:],
                        )
                    xt_sb = work.tile([P, DC, P], bf16, tag="xt", bufs=3)
                    nc.vector.tensor_copy(out=xt_sb[:], in_=tp[:])
                    xts[b] = xt_sb
                es = {}
                for b in range(NB):
                    s_ps = pb.tile([P, OS], f32, tag="sps")
                    for c in range(DC):
                        nc.tensor.matmul(
                            out=s_ps[:], lhsT=xts[b][:, c, :],
                            rhs=aw_bf[:, c, :],
                            start=(c == 0), stop=(c == DC - 1),
                        )
                    e_sb = work.tile([P, OS], bf16, tag="e", bufs=4)
                    nc.scalar.activation(out=e_sb[:], in_=s_ps[:], func=AF.Exp)
                    es[b] = e_sb
                if prev is not None:
                    emit_xp(prev)
                prev = (i, es)
                if i == 8:
                    # candidate-row gathers on the otherwise-idle SWDGE path
                    for gb in range(NB):
                        for cc in range(CC):
                            bf_t = gat.tile([P, LAB], bf16, tag="blvf",
                                            name=f"blvf_{gb}_{cc}")
                            nc.gpsimd.indirect_dma_start(
                                out=bf_t[:], out_offset=None, in_=lab[:, :],
                                in_offset=bass.IndirectOffsetOnAxis(
                                    ap=cand_sb[:, gb, cc:cc + 1], axis=0,
                                ),
                            )
                            blv_f[gb, cc] = bf_t
                if i == 13:
                    # bulk h-weights land right before phase B needs them,
                    # queued on the sync ring behind the X stream
                    for j in range(4):
                        nc.sync.dma_start(
                            out=hw_bf[:, j * 2:(j + 1) * 2, :],
                            in_=hw[j * 2 * P:(j + 1) * 2 * P, :].rearrange(
                                "(c p) h -> p c h", p=P
                            ),
                        )
            emit_xp(prev)

            # candidate-vector transposes go first on the PE queue: they only
            # need the gathers, so the PE stays busy while the phase-B
            # normalize chain (reciprocal -> scale -> evac) runs on DVE/ACT
            blvT = {}
            for b in range(NB):
                bt = work.tile([P, HC, NCC], bf16, tag=f"blvT{b}")
                for cc in range(CC):
                    tp4 = pa.tile([P, HC, P], bf16, tag="tp")
                    for hc in range(HC):
                        nc.tensor.transpose(
                            out=tp4[:, hc, :],
                            in_=blv_f[b, cc][:, hc * P:(hc + 1) * P],
                            identity=ident[:],
                        )
                    nc.vector.tensor_copy(
                        out=bt[:, :, cc * P:(cc + 1) * P], in_=tp4[:]
                    )
                blvT[b] = bt

            # ======== phase B: normalize + project, both batches stacked =====
            rzA = work.tile([P, 1], f32, tag="rzA")
            nc.vector.reciprocal(out=rzA[:], in_=zz2[:])
            xp2 = work.tile([P, D], bf16, tag="xp2")
            for nh in range(2):
                nc.scalar.activation(
                    out=xp2[:, nh * 512:(nh + 1) * 512],
                    in_=xpu2[:, nh * 512:(nh + 1) * 512],
                    func=AF.Copy, scale=rzA[:],
                )
            # Xp^T (d on partitions); evacuate in halves so the projection
            # matmuls start after the first four chunks
            xpt2 = work.tile([P, DC, P], bf16, tag="xpt2")
            tp2 = pa.tile([P, DC, P], bf16, tag="tp")
            for c in range(DC):
                nc.tensor.transpose(
                    out=tp2[:, c, :], in_=xp2[:, c * P:(c + 1) * P],
                    identity=ident[:],
                )
                if c == DC // 2 - 1:
                    nc.vector.tensor_copy(
                        out=xpt2[:, :DC // 2, :], in_=tp2[:, :DC // 2, :]
                    )
            nc.vector.tensor_copy(
                out=xpt2[:, DC // 2:, :], in_=tp2[:, DC // 2:, :]
            )
            xpf2 = work.tile([P, LAB], bf16, tag="xpf2")
            for nh in range(2):
                xph = pb.tile([P, 512], f32, tag="sps")
                for c in range(DC):
                    nc.tensor.matmul(
                        out=xph[:], lhsT=xpt2[:, c, :],
                        rhs=hw_bf[:, c, nh * 512:(nh + 1) * 512],
                        start=(c == 0), stop=False,
                    )
                nc.tensor.matmul(
                    out=xph[:], lhsT=ones_one[:],
                    rhs=hb_bf[:, nh * 512:(nh + 1) * 512],
                    start=False, stop=True,
                )
                nc.scalar.activation(
                    out=xpf2[:, nh * 512:(nh + 1) * 512], in_=xph[:],
                    func=AF.Relu,
                )
            # Xpf^T (h on partitions) for the candidate scores
            xpft2 = work.tile([P, HC, P], bf16, tag="xpft2")
            tp3 = pa.tile([P, HC, P], bf16, tag="tp")
            for hc in range(HC):
                nc.tensor.transpose(
                    out=tp3[:, hc, :], in_=xpf2[:, hc * P:(hc + 1) * P],
                    identity=ident[:],
                )
                if hc == HC // 2 - 1:
                    nc.vector.tensor_copy(
                        out=xpft2[:, :HC // 2, :], in_=tp3[:, :HC // 2, :]
                    )
            nc.vector.tensor_copy(
                out=xpft2[:, HC // 2:, :], in_=tp3[:, HC // 2:, :]
            )

            # ======== phase C/D interleaved per candidate chunk ========
            # deferred softmax over OS: out_unnorm = E2^T Xpf, scaled by
            # 1/rowsum at PSUM evacuation
            e2t2 = work.tile([P, CC, P], bf16, tag="e2t2")
            tp5 = pa.tile([P, CC, P], bf16, tag="tp")
            rz2s = {}
            for cc in range(CC):
                for b in range(NB):
                    s2 = pb.tile([P, OS], f32, tag="sps")
                    for hc in range(HC):
                        nc.tensor.matmul(
                            out=s2[:],
                            lhsT=blvT[b][:, hc, cc * P:(cc + 1) * P],
                            rhs=xpft2[:, hc, b * OS:(b + 1) * OS],
                            start=(hc == 0), stop=(hc == HC - 1),
                        )
                    negm = work.tile([P, 1], f32, tag="negm")
                    nc.vector.tensor_reduce(
                        out=negm[:], in_=s2[:], axis=mybir.AxisListType.X,
                        op=mybir.AluOpType.max, negate=True,
                    )
                    e2 = work.tile([P, OS], bf16, tag="e2")
                    sume = work.tile([P, 1], f32, tag="sume")
                    nc.scalar.activation(
                        out=e2[:], in_=s2[:], func=AF.Exp, bias=negm[:],
                        accum_out=sume[:],
                    )
                    rz2 = work.tile([P, 1], f32, tag="rz2", bufs=4,
                                    name=f"rz2_{b}_{cc}")
                    nc.vector.reciprocal(out=rz2[:], in_=sume[:])
                    rz2s[b, cc] = rz2
                    # E2^T lands on partitions b*64..: batch1 goes to PE
                    # column group (0, 64); evacuate per quadrant so phase D
                    # for this cc can start immediately
                    nc.tensor.transpose(
                        out=tp5[b * OS:(b + 1) * OS, cc, :], in_=e2[:],
                        identity=ident[:],
                    )
                    nc.vector.tensor_copy(
                        out=e2t2[b * OS:(b + 1) * OS, cc, :],
                        in_=tp5[b * OS:(b + 1) * OS, cc, :],
                    )
            # out = softmax(s2)^T Xpf: K=64 row-tiled pairs, b0 on PE
            # rows 0-63, b1 on rows 64-127; the softmax chains for later
            # chunks drain on DVE/ACT underneath these matmuls
            for cc in range(CC):
                for nh in range(2):
                    ops = {}
                    for b in range(NB):
                        op = pb.tile([P, 512], f32, tag="sps")
                        nc.tensor.matmul(
                            out=op[:],
                            lhsT=e2t2[b * OS:(b + 1) * OS, cc, :],
                            rhs=xpf2[b * OS:(b + 1) * OS,
                                     nh * 512:(nh + 1) * 512],
                            start=True, stop=True,
                        )
                        ops[b] = op
                    for b in range(NB):
                        ob = outp.tile([P, 512], bf16, tag="ob")
                        if b == 0:
                            nc.scalar.activation(
                                out=ob[:], in_=ops[b][:], func=AF.Copy,
                                scale=rz2s[b, cc][:],
                            )
                        else:
                            nc.vector.tensor_scalar(
                                out=ob[:], in0=ops[b][:],
                                scalar1=rz2s[b, cc][:],
                                scalar2=None, op0=mybir.AluOpType.mult,
                            )
                        nc.sync.dma_start(
                            out=out_d[b, cc * P:(cc + 1) * P,
                                      nh * 512:(nh + 1) * 512],
                            in_=ob[:],
                        )
    nc.finalize()
    return nc


def _ensure_neuron_platform():
    # The kernel must execute on the axon-tunneled NeuronCores; a stray
    # JAX_PLATFORMS=cpu pin (common for running the jax reference) would
    # hide them from PJRT. Only act if jax hasn't initialized a backend yet.
    import os
    import sys

    if os.environ.get("JAX_PLATFORMS") == "cpu":
        jax = sys.modules.get("jax")
        initialized = False
        if jax is not None:
            try:
                from jax._src import xla_bridge

                initialized = xla_bridge.backends_are_initialized()
            except Exception:
                initialized = False
        if not initialized:
            del os.environ["JAX_PLATFORMS"]


def _get_program():
    global _PROG
    if _PROG is None:
        _ensure_neuron_platform()
        _PROG = _build_program()
    return _PROG


def _make_in_maps(inputs):
    import ml_dtypes

    bf16 = ml_dtypes.bfloat16
    X = np.ascontiguousarray(
        np.asarray(inputs["X"], dtype=np.float32).astype(bf16)
    )
    cand = np.ascontiguousarray(
        np.asarray(inputs["candidate"]).astype(np.int32)
    )
    a_w = np.asarray(inputs["a_w"], dtype=np.float32)
    h_w = np.asarray(inputs["h_w"], dtype=np.float32)
    h_b = np.asarray(inputs["h_b"], dtype=np.float32)
    g = np.asarray(inputs["bn_gamma"], dtype=np.float32)
    be = np.asarray(inputs["bn_beta"], dtype=np.float32)
    mu = np.asarray(inputs["bn_mean"], dtype=np.float32)
    var = np.asarray(inputs["bn_var"], dtype=np.float32)
    lab = np.ascontiguousarray(
        np.asarray(inputs["labDescVec"], dtype=np.float32).astype(bf16)
    )

    s = g / np.sqrt(var + BN_EPS)
    hw_eff = np.ascontiguousarray((h_w * s[None, :]).astype(bf16))
    hb_eff = ((h_b - mu) * s + be).astype(bf16)
    aw_bf = a_w.astype(bf16)

    in_maps = []
    for ci in range(N_CORES):
        in_maps.append({
            "X": X[ci * NB:(ci + 1) * NB],
            "cand": cand[ci * NB:(ci + 1) * NB],
            "aw": aw_bf,
            "hw": hw_eff,
            "hb": hb_eff,
            "lab": lab,
        })
    return in_maps


def run(inputs, trace=False, tmpdir=None):
    from concourse.bass_utils import run_bass_kernel_spmd

    nc = _get_program()
    in_maps = _make_in_maps(inputs)
    kwargs = {}
    if trace and tmpdir is None:
        tmpdir = "/root/problem/trace_out"
        import os
        import shutil

        shutil.rmtree(tmpdir, ignore_errors=True)
        os.makedirs(tmpdir, exist_ok=True)
    if tmpdir is not None:
        kwargs["tmpdir"] = tmpdir
    res = run_bass_kernel_spmd(
        nc, in_maps, list(range(N_CORES)), trace=trace, **kwargs,
    )
    out = np.concatenate(
        [np.asarray(r["out"]).astype(np.float32) for r in res.results], axis=0
    )
    return out, res


def kernel(**inputs):
    out, _ = run(inputs, trace=False)
    return out
